# revision 2
# baseline (speedup 1.0000x reference)
"""Batch-hard triplet loss on 8 Trainium2 NeuronCores.

Math (matches the reference exactly up to fp rounding):
  d_ij   = ||h_i||^2 + ||h_j||^2 - 2 h_i.h_j, clamped to [EPS, inf)
  hp_i   = max over j (same label, j != i) of d_ij
  hn_i   = 2nd-smallest over j (different label) of d_ij
  loss_i = max(hp_i - hn_i + ALPHA, 0)
  out    = sum(loss_i[loss_i > EPS]) / count(loss_i > EPS)

Device strategy: rows are sharded over 8 cores (1024 each). Each core runs
one augmented GEMM whose PSUM output is directly the mining quantity

  p_ij = 2 h_i.h_j - ||h_j||^2 - BIG * [label_i == label_j]

built from a K = D + C + 2 = 1154 contraction:
  a_i = ( 2 h_i,  -BIG * onehot(label_i),  -1, -1 )
  b_j = ( h_j,     onehot(label_j),        xnorm_hi_j, xnorm_lo_j )

Row-constant terms (||h_i||^2, the EPS clamp) cancel in hp - hn, so they are
never computed.  With t_ij := d_ij - ||h_i||^2 = -p_ij - BIG*eq:
  hp_i = -min_j(p_ij) - BIG        (positives carry -BIG, dominate the min;
                                    Sterbenz: the BIG subtraction is exact)
  hn_i = -max8(p_i)[1]             (negatives are the largest p; the DVE Max8
                                    instruction gives the top-8 descending, so
                                    element 1 is the 2nd-smallest distance,
                                    with tie multiplicity matching top_k)
  loss_i = max( max8[1] - min + (ALPHA - BIG), 0 )

The masked mean over all 8192 rows is done on the host from the returned
per-row loss vectors (8 x 1024 floats).
"""

import functools

import numpy as np
import ml_dtypes

import concourse.bacc as bacc
import concourse.tile as tile
from concourse import mybir
from concourse.bass_utils import run_bass_kernel_spmd

BF16 = mybir.dt.bfloat16
F32 = mybir.dt.float32

N, D, C = 8192, 1024, 128
NCORES = 8
P = 128
JB = 512  # matmul moving free dim = one fp32 PSUM bank
ALPHA = 0.1
EPS = 1e-7
BIG = 8192.0


def build_program(rows, n, d, c, jb, psum_bufs=8, b_bufs=2):
    """Emit the per-core Bass/Tile program (identical on all cores)."""
    ka = d + c + 2
    kh = d // P
    m_chunks = rows // P
    nj = n // jb
    assert rows % P == 0 and d % P == 0 and n % jb == 0 and c <= P

    nc = bacc.Bacc("TRN2", target_bir_lowering=False)
    A = nc.dram_tensor("A", [ka, rows], BF16, kind="ExternalInput")
    B = nc.dram_tensor("B", [ka, n], BF16, kind="ExternalInput")
    loss = nc.dram_tensor("loss", [rows], F32, kind="ExternalOutput")

    with tile.TileContext(nc) as tc:
        with (
            tc.tile_pool(name="apool", bufs=1) as apool,
            tc.tile_pool(name="bpool", bufs=b_bufs) as bpool,
            tc.tile_pool(name="psum", bufs=psum_bufs, space="PSUM") as pp,
            tc.tile_pool(name="mpool", bufs=1) as mpool,
            tc.tile_pool(name="fpool", bufs=2) as fpool,
        ):
            # A (lhsT) stays resident in SBUF for the whole kernel.
            a_h = []
            for kc in range(kh):
                t = apool.tile([P, rows], BF16, tag=f"ah{kc}")
                nc.sync.dma_start(out=t[:], in_=A[kc * P:(kc + 1) * P, :])
                a_h.append(t)
            a_oh = apool.tile([c, rows], BF16, tag="aoh")
            nc.sync.dma_start(out=a_oh[:], in_=A[d:d + c, :])
            a_nm = apool.tile([2, rows], BF16, tag="anm")
            nc.sync.dma_start(out=a_nm[:], in_=A[d + c:ka, :])

            # Per-row-chunk partial mining results, merged after the j loop.
            v8 = [mpool.tile([P, nj * 8], F32, tag=f"v8_{m}", name=f"v8_{m}")
                  for m in range(m_chunks)]
            gmin = [mpool.tile([P, nj], F32, tag=f"gm_{m}", name=f"gmin_{m}")
                    for m in range(m_chunks)]

            for j in range(nj):
                js = slice(j * jb, (j + 1) * jb)
                bts = []
                for kc in range(kh):
                    bt = bpool.tile([P, jb], BF16, tag=f"b{kc}")
                    nc.sync.dma_start(out=bt[:], in_=B[kc * P:(kc + 1) * P, js])
                    bts.append(bt)
                b_oh = bpool.tile([c, jb], BF16, tag="boh")
                nc.sync.dma_start(out=b_oh[:], in_=B[d:d + c, js])
                b_nm = bpool.tile([2, jb], BF16, tag="bnm")
                nc.sync.dma_start(out=b_nm[:], in_=B[d + c:ka, js])

                for m in range(m_chunks):
                    ms = slice(m * P, (m + 1) * P)
                    ps = pp.tile([P, jb], F32, name="ps")
                    for kc in range(kh):
                        nc.tensor.matmul(ps[:], a_h[kc][:, ms], bts[kc][:],
                                         start=(kc == 0), stop=False)
                    nc.tensor.matmul(ps[:], a_oh[:, ms], b_oh[:],
                                     start=False, stop=False)
                    nc.tensor.matmul(ps[:], a_nm[:, ms], b_nm[:],
                                     start=False, stop=True)
                    nc.vector.max(v8[m][:, j * 8:(j + 1) * 8], ps[:])
                    nc.vector.tensor_reduce(gmin[m][:, j:j + 1], ps[:],
                                            axis=mybir.AxisListType.X,
                                            op=mybir.AluOpType.min)

            for m in range(m_chunks):
                vf = fpool.tile([P, 8], F32, tag="vf")
                nc.vector.max(vf[:], v8[m][:])
                gm = fpool.tile([P, 1], F32, tag="gm")
                nc.vector.tensor_reduce(gm[:], gmin[m][:],
                                        axis=mybir.AxisListType.X,
                                        op=mybir.AluOpType.min)
                lo = fpool.tile([P, 1], F32, tag="lo")
                nc.vector.tensor_sub(lo[:], vf[:, 1:2], gm[:])
                nc.vector.tensor_scalar(out=lo[:], in0=lo[:],
                                        scalar1=float(ALPHA - BIG), scalar2=0.0,
                                        op0=mybir.AluOpType.add,
                                        op1=mybir.AluOpType.max)
                nc.sync.dma_start(out=loss[m * P:(m + 1) * P], in_=lo[:, 0:1])

    nc.compile()
    return nc


def make_inputs(H, labels, n, d, c, ncores):
    """Host-side packing of the augmented GEMM operands (bf16)."""
    H = np.ascontiguousarray(np.asarray(H, dtype=np.float32))
    labels = np.asarray(labels).astype(np.int64).ravel()
    ka = d + c + 2
    rows = n // ncores

    Hb = H.astype(ml_dtypes.bfloat16)
    Hb32 = Hb.astype(np.float32)
    xn = np.einsum("ij,ij->i", Hb32.astype(np.float64), Hb32.astype(np.float64))
    xn_hi = xn.astype(ml_dtypes.bfloat16)
    xn_lo = (xn - xn_hi.astype(np.float64)).astype(ml_dtypes.bfloat16)
    oh = labels[None, :] == np.arange(c, dtype=np.int64)[:, None]  # [c, n]

    Bm = np.empty((ka, n), dtype=ml_dtypes.bfloat16)
    Bm[:d] = Hb.T
    Bm[d:d + c] = oh.astype(ml_dtypes.bfloat16)
    Bm[d + c] = xn_hi
    Bm[d + c + 1] = xn_lo

    in_maps = []
    for cix in range(ncores):
        sl = slice(cix * rows, (cix + 1) * rows)
        Am = np.empty((ka, rows), dtype=ml_dtypes.bfloat16)
        Am[:d] = (2.0 * Hb32[sl].T).astype(ml_dtypes.bfloat16)  # exact: 2*bf16
        Am[d:d + c] = (-BIG * oh[:, sl]).astype(ml_dtypes.bfloat16)
        Am[d + c:ka] = -1.0
        in_maps.append({"A": Am, "B": Bm})
    return in_maps


@functools.lru_cache(maxsize=1)
def _get_program():
    return build_program(N // NCORES, N, D, C, JB)


def _finalize(loss_rows):
    loss_all = np.concatenate([np.asarray(l, dtype=np.float64) for l in loss_rows])
    rel = loss_all > EPS
    cnt = int(rel.sum())
    if cnt == 0:
        return np.float32(np.nan)
    return np.float32(loss_all[rel].sum() / cnt)


def kernel(H, labels):
    in_maps = make_inputs(H, labels, N, D, C, NCORES)
    res = run_bass_kernel_spmd(_get_program(), in_maps, list(range(NCORES)))
    return _finalize([r["loss"] for r in res.results])
